# revision 4
# baseline (speedup 1.0000x reference)
# CWVAE (3-level RSSM scan) Trainium2 kernel — single NeuronCore.
#
# Strategy:
#  * All matmuls bf16 x bf16 -> fp32 PSUM. Batch (B=32) rides the PE stationary
#    operand; weights stream. 4x column tiling (128x32 tiles) fills the array.
#  * Activations live in "quartered" layout: SBUF [128, 256] where partition
#    32*q + b holds hidden dims [256q, 256q+256) of batch sample b.
#  * PE transposes (identity matmul) produce the [K,32] lhsT blocks needed by
#    the next matmul in the recurrence.
#  * qmean folded into next step's h1 via W_fuse = W_ps @ qm_w (host-computed),
#    so the carried state is (qh, det) and qmean is recovered in the postpass.
#  * obs/context contributions to h1/qh are precomputed outside the scan
#    (obs_part / c_part) as M-batched matmuls; heads (pmean/pstd/qmean/qstd)
#    are computed in an M-batched postpass from stored transposed det/qh.
import numpy as np
import ml_dtypes
from contextlib import ExitStack

import concourse.bass as bass
import concourse.tile as tile
from concourse import mybir
from concourse.masks import make_identity

F32 = mybir.dt.float32
BF16 = mybir.dt.bfloat16
NBF = ml_dtypes.bfloat16

B = 32
D = 1024          # deter
S = 256           # stoch
E = 1024          # emb
NQ = 4            # quarters
QD = D // NQ      # 256
KB = D // 128     # 8 K-blocks of the 1024-dim contractions
MIN_STD = 1e-4
SP_BIAS = 0.54


def bfc(x):
    return np.ascontiguousarray(x.astype(NBF))


def pack_quartered(WT):
    """WT: [K, N] (K contraction, N output) -> [K//128, NQ, 128, N//NQ]
    tile (k, j) = WT[128k:128k+128, (N//NQ)*j : (N//NQ)*(j+1)]"""
    K, N = WT.shape
    nj = N // NQ
    out = np.empty((K // 128, NQ, 128, nj), WT.dtype)
    for k in range(K // 128):
        for j in range(NQ):
            out[k, j] = WT[128 * k:128 * (k + 1), nj * j:nj * (j + 1)]
    return np.ascontiguousarray(out)


def prep_inputs(inputs, T0=64):
    """Host-side: cast/permute weights into SBUF tile layouts. Returns dict."""
    Ts = [T0, T0 // 4, T0 // 16]
    d = {}
    for l in range(3):
        ph1 = inputs["ph1_w"][l].astype(np.float32)       # [E, S+D]
        W_ps = ph1[:, :S]                                  # [E, S]
        W_ctx = ph1[:, S:]                                 # [E, D]
        qm = inputs["qmean_w"][l].astype(np.float32)       # [S, E]
        W_fuse = (W_ps.astype(np.float64) @ qm.astype(np.float64)).astype(np.float32)  # [E, E]
        wihT = inputs["gru_wih"][l].astype(np.float32).T   # [E, 3D]
        whhT = inputs["gru_whh"][l].astype(np.float32).T   # [D, 3D]
        wqdT = inputs["qh1_w"][l][:, :D].astype(np.float32).T    # [D, E]
        wqoT = inputs["qh1_w"][l][:, D:].astype(np.float32).T    # [E(obs), E]
        wctxT = W_ctx.T                                    # [D, E]
        wfuseT = W_fuse.T                                  # [E(qh), E(h1)]

        def rz(WT):  # [K, 3D] -> rz tiles [K//128, 4, 128, 512]
            K = WT.shape[0]
            out = np.empty((K // 128, NQ, 128, 2 * QD), np.float32)
            for k in range(K // 128):
                for j in range(NQ):
                    out[k, j, :, :QD] = WT[128 * k:128 * (k + 1), QD * j:QD * (j + 1)]
                    out[k, j, :, QD:] = WT[128 * k:128 * (k + 1), D + QD * j:D + QD * (j + 1)]
            return out

        def ngate(WT):
            K = WT.shape[0]
            out = np.empty((K // 128, NQ, 128, QD), np.float32)
            for k in range(K // 128):
                for j in range(NQ):
                    out[k, j] = WT[128 * k:128 * (k + 1), 2 * D + QD * j:2 * D + QD * (j + 1)]
            return out

        d[f"wihrz{l}"] = bfc(rz(wihT))
        d[f"wihn{l}"] = bfc(ngate(wihT))
        d[f"whhrz{l}"] = bfc(rz(whhT))
        d[f"whhn{l}"] = bfc(ngate(whhT))
        d[f"wqd{l}"] = bfc(pack_quartered(wqdT))
        d[f"wfuse{l}"] = bfc(pack_quartered(wfuseT))
        d[f"wqo{l}"] = bfc(np.ascontiguousarray(wqoT.reshape(KB, 128, E)))
        if l < 2:
            d[f"wctx{l}"] = bfc(pack_quartered(wctxT))
        obs = inputs[f"obs_l{l}"].astype(np.float32)       # [B, T, E]
        d[f"obs{l}"] = bfc(obs)
    # postpass heads, packed as one [4, 8, 128, 256] (head, k, p, n): pm, ps, qm, qs
    post = np.stack([
        np.ascontiguousarray(inputs["pmean_w"][0].astype(np.float32).T.reshape(KB, 128, S)),
        np.ascontiguousarray(inputs["pstd_w"][0].astype(np.float32).T.reshape(KB, 128, S)),
        np.ascontiguousarray(inputs["qmean_w"][0].astype(np.float32).T.reshape(KB, 128, S)),
        np.ascontiguousarray(inputs["qstd_w"][0].astype(np.float32).T.reshape(KB, 128, S)),
    ])
    d["wpost"] = bfc(post)
    return d


INPUT_SPECS = None  # filled by build()


def lhs_blk(tT, k):
    """transposed-activation SBUF tile [128, 2, 128] -> lhsT block k [128, 32]"""
    s, q = k % 2, k // 2
    return tT[:, s, 32 * q:32 * (q + 1)]


def build(ctx: ExitStack, tc: tile.TileContext, outs, ins, T0=64):
    nc = tc.nc
    Ts = [T0, T0 // 4, T0 // 16]
    out_f = outs["out_f"]

    const = ctx.enter_context(tc.tile_pool(name="const", bufs=1))
    wpool = ctx.enter_context(tc.tile_pool(name="wpool", bufs=1))
    cpool = ctx.enter_context(tc.tile_pool(name="cpool", bufs=2))
    spool = ctx.enter_context(tc.tile_pool(name="spool", bufs=3))
    work = ctx.enter_context(tc.tile_pool(name="work", bufs=2))
    pps = ctx.enter_context(tc.tile_pool(name="pps", bufs=1, space="PSUM"))
    dpool = ctx.enter_context(tc.tile_pool(name="dpool", bufs=1, space="DRAM"))

    ident = const.tile([128, 128], BF16)
    make_identity(nc, ident)
    sp_bias = const.tile([128, 1], F32)
    nc.vector.memset(sp_bias, SP_BIAS)

    # DRAM intermediates (via tracked DRAM pool tiles)
    obs_part_d = [dpool.tile([Ts[l], 128, QD], F32, tag=f"obs_part{l}", name=f"obs_part{l}")
                  for l in range(3)]
    c_part_d = {l: dpool.tile([Ts[l], 128, QD], F32, tag=f"c_part{l}", name=f"c_part{l}")
                for l in range(2)}
    detT_d = [dpool.tile([Ts[l], 128, 2, 128], BF16, tag=f"detT{l}", name=f"detT{l}")
              for l in range(3)]
    qhT_d = dpool.tile([Ts[0], 128, 2, 128], BF16, tag="qhT0", name="qhT0")

    # ---------- obs_part prelude (all levels) ----------
    for l in (2, 1, 0):
        wqo = wpool.tile([128, KB, E], BF16, tag="big1", name=f"wqo{l}")
        for k in range(KB):
            nc.gpsimd.dma_start(out=wqo[:, k, :], in_=ins[f"wqo{l}"][k])
        obs_flat = ins[f"obs{l}"].flatten_outer_dims()      # [B*T, E]
        R = B * Ts[l]
        Tl = Ts[l]
        for m in range((R + 127) // 128):
            mrows = min(128, R - 128 * m)
            obsT_m = work.tile([128, KB, 128], BF16, tag="obsT")
            for k in range(KB):
                nc.sync.dma_start_transpose(
                    out=obsT_m[:, k, :mrows],
                    in_=obs_flat[128 * m:128 * m + mrows, 128 * k:128 * (k + 1)])
            p0 = pps.tile([128, 512], F32, tag="p_rz", name="p0")
            p1 = pps.tile([128, 512], F32, tag="p_h1", name="p1")
            for k in range(KB):
                nc.tensor.matmul(p0[:mrows], obsT_m[:, k, :mrows], wqo[:, k, 0:512],
                                 start=(k == 0), stop=(k == KB - 1))
                nc.tensor.matmul(p1[:mrows], obsT_m[:, k, :mrows], wqo[:, k, 512:1024],
                                 start=(k == 0), stop=(k == KB - 1))
            osb = work.tile([128, E], F32, tag="opart_sb")
            nc.vector.tensor_copy(osb[:mrows, 0:512], p0[:mrows])
            nc.scalar.copy(osb[:mrows, 512:1024], p1[:mrows])
            # rows of this M-tile: row r = (b - b_base)*T + t, b_base = 128m // T
            nb = mrows // Tl
            b_base = (128 * m) // Tl
            dest = obs_part_d[l].rearrange("t (q b) f -> b t q f", q=NQ)
            osb_v = osb.rearrange("p (q f) -> p q f", q=NQ)
            for bb in range(nb):
                nc.sync.dma_start(out=dest[b_base + bb],
                                  in_=osb_v[Tl * bb:Tl * (bb + 1)])

    # ---------- level loop ----------
    for l in (2, 1, 0):
        T = Ts[l]
        top = (l == 2)
        # c_part phase for this level (from parent's stored detT)
        if not top:
            wctx = wpool.tile([128, KB, NQ, QD], BF16, tag="big1", name=f"wctx{l}")
            for k in range(KB):
                nc.gpsimd.dma_start(out=wctx[:, k], in_=ins[f"wctx{l}"][k].rearrange("j p n -> p j n"))
            for p in range(Ts[l + 1]):
                plhsT = work.tile([128, 2, 128], BF16, tag="pstep_lhsT")
                nc.sync.dma_start(out=plhsT, in_=detT_d[l + 1][p])
                pq = pps.tile([128, QD], F32, tag="p_qh", name="pq_cp")
                for j in range(NQ):
                    for k in range(KB):
                        nc.tensor.matmul(pq[32 * j:32 * (j + 1), :], lhs_blk(plhsT, k),
                                         wctx[:, k, j, :], start=(k == 0), stop=(k == KB - 1),
                                         tile_position=(0, 32 * j))
                csb = work.tile([128, QD], F32, tag="cpart_sb")
                nc.vector.tensor_copy(csb, pq)
                for i in range(4):
                    nc.sync.dma_start(out=c_part_d[l][4 * p + i], in_=csb)

        # level weights
        wihrz = wpool.tile([128, KB, NQ, 2 * QD], BF16, tag="wihrz", name=f"wihrz{l}")
        whhrz = wpool.tile([128, KB, NQ, 2 * QD], BF16, tag="whhrz", name=f"whhrz{l}")
        wihn = wpool.tile([128, KB, NQ, QD], BF16, tag="wihn", name=f"wihn{l}")
        whhn = wpool.tile([128, KB, NQ, QD], BF16, tag="whhn", name=f"whhn{l}")
        wqd = wpool.tile([128, KB, NQ, QD], BF16, tag="wqd", name=f"wqd{l}")
        wfuse = wpool.tile([128, KB, NQ, QD], BF16, tag="wfuse", name=f"wfuse{l}")
        for k in range(KB):
            nc.gpsimd.dma_start(out=wihrz[:, k], in_=ins[f"wihrz{l}"][k].rearrange("j p n -> p j n"))
            nc.gpsimd.dma_start(out=whhrz[:, k], in_=ins[f"whhrz{l}"][k].rearrange("j p n -> p j n"))
            nc.gpsimd.dma_start(out=wihn[:, k], in_=ins[f"wihn{l}"][k].rearrange("j p n -> p j n"))
            nc.gpsimd.dma_start(out=whhn[:, k], in_=ins[f"whhn{l}"][k].rearrange("j p n -> p j n"))
            nc.gpsimd.dma_start(out=wqd[:, k], in_=ins[f"wqd{l}"][k].rearrange("j p n -> p j n"))
            nc.gpsimd.dma_start(out=wfuse[:, k], in_=ins[f"wfuse{l}"][k].rearrange("j p n -> p j n"))

        detf_c = detT_c = qhT_c = None
        for t in range(T):
            first = (t == 0)
            has_ctx = not top
            # stream tiles
            cpt = None
            if has_ctx:
                cpt = spool.tile([128, QD], F32, tag="cpt")
                nc.sync.dma_start(out=cpt, in_=c_part_d[l][t])
            opt = spool.tile([128, QD], F32, tag="opt")
            nc.sync.dma_start(out=opt, in_=obs_part_d[l][t])

            # --- MM-A: h1 pre-activation from qh carry (fused weights)
            ph1 = None
            if not first:
                ph1 = pps.tile([128, QD], F32, tag="p_h1", name="ph1")
                for j in range(NQ):
                    for k in range(KB):
                        nc.tensor.matmul(ph1[32 * j:32 * (j + 1), :], lhs_blk(qhT_c, k),
                                         wfuse[:, k, j, :], start=(k == 0), stop=(k == KB - 1),
                                         tile_position=(0, 32 * j))
            # --- h1 (bf16, relu)
            h1bf = None
            if ph1 is not None and cpt is not None:
                h1f = work.tile([128, QD], F32, tag="h1f")
                nc.vector.tensor_add(h1f, ph1, cpt)
                h1bf = work.tile([128, QD], BF16, tag="h1bf")
                nc.vector.tensor_scalar_max(h1bf, h1f, 0.0)
            elif ph1 is not None:
                h1bf = work.tile([128, QD], BF16, tag="h1bf")
                nc.vector.tensor_scalar_max(h1bf, ph1, 0.0)
            elif cpt is not None:
                h1bf = work.tile([128, QD], BF16, tag="h1bf")
                nc.vector.tensor_scalar_max(h1bf, cpt, 0.0)
            # --- T(h1)
            h1T = None
            if h1bf is not None:
                h1T = work.tile([128, 2, 128], BF16, tag="h1T")
                for s in range(2):
                    pt = pps.tile([128, 128], BF16, tag="p_t", bufs=2, name="pt_h1")
                    nc.tensor.transpose(pt, h1bf[:, 128 * s:128 * (s + 1)], ident)
                    nc.vector.tensor_copy(h1T[:, s, :], pt)

            # --- GRU
            do_gh = not first
            do_gi = h1T is not None
            detf_new = cpool.tile([128, QD], F32, tag="detf")
            detbf = None
            if do_gh or do_gi:
                prz = pps.tile([128, 2 * QD], F32, tag="p_rz", name="prz")
                pgin = pps.tile([128, QD], F32, tag="p_gin", name="pgin")
                pghn = pps.tile([128, QD], F32, tag="p_ghn", name="pghn")
                if do_gh:
                    for j in range(NQ):
                        for k in range(KB):
                            nc.tensor.matmul(prz[32 * j:32 * (j + 1), :], lhs_blk(detT_c, k),
                                             whhrz[:, k, j, :], start=(k == 0),
                                             stop=(k == KB - 1) and not do_gi,
                                             tile_position=(0, 32 * j))
                    for j in range(NQ):
                        for k in range(KB):
                            nc.tensor.matmul(pghn[32 * j:32 * (j + 1), :], lhs_blk(detT_c, k),
                                             whhn[:, k, j, :], start=(k == 0), stop=(k == KB - 1),
                                             tile_position=(0, 32 * j))
                if do_gi:
                    for j in range(NQ):
                        for k in range(KB):
                            nc.tensor.matmul(prz[32 * j:32 * (j + 1), :], lhs_blk(h1T, k),
                                             wihrz[:, k, j, :], start=(k == 0) and not do_gh,
                                             stop=(k == KB - 1),
                                             tile_position=(0, 32 * j))
                    for j in range(NQ):
                        for k in range(KB):
                            nc.tensor.matmul(pgin[32 * j:32 * (j + 1), :], lhs_blk(h1T, k),
                                             wihn[:, k, j, :], start=(k == 0), stop=(k == KB - 1),
                                             tile_position=(0, 32 * j))
                # gates
                r_s = work.tile([128, QD], F32, tag="r_s")
                nc.scalar.activation(r_s, prz[:, 0:QD], mybir.ActivationFunctionType.Sigmoid)
                if do_gh and do_gi:
                    t1 = work.tile([128, QD], F32, tag="t1")
                    nc.vector.tensor_mul(t1, r_s, pghn)
                    t2 = work.tile([128, QD], F32, tag="t2")
                    nc.vector.tensor_add(t2, t1, pgin)
                    n_in = t2
                elif do_gi:
                    n_in = pgin
                else:
                    t1 = work.tile([128, QD], F32, tag="t1")
                    nc.vector.tensor_mul(t1, r_s, pghn)
                    n_in = t1
                n_s = work.tile([128, QD], F32, tag="n_s")
                nc.scalar.activation(n_s, n_in, mybir.ActivationFunctionType.Tanh)
                z_s = work.tile([128, QD], F32, tag="z_s")
                nc.scalar.activation(z_s, prz[:, QD:2 * QD], mybir.ActivationFunctionType.Sigmoid)
                d1 = work.tile([128, QD], F32, tag="d1")
                if not first:
                    nc.vector.tensor_sub(d1, detf_c, n_s)
                    d2 = work.tile([128, QD], F32, tag="d2")
                    nc.vector.tensor_mul(d2, d1, z_s)
                    nc.vector.tensor_add(detf_new, n_s, d2)
                else:
                    nc.vector.tensor_mul(d1, z_s, n_s)
                    nc.vector.tensor_sub(detf_new, n_s, d1)
                detbf = work.tile([128, QD], BF16, tag="detbf")
                nc.vector.tensor_copy(detbf, detf_new)
            else:
                nc.vector.memset(detf_new, 0.0)

            # --- T(det)
            detT_new = cpool.tile([128, 2, 128], BF16, tag="detT")
            if detbf is not None:
                for s in range(2):
                    pt = pps.tile([128, 128], BF16, tag="p_t", bufs=2, name="pt_d")
                    nc.tensor.transpose(pt, detbf[:, 128 * s:128 * (s + 1)], ident)
                    nc.vector.tensor_copy(detT_new[:, s, :], pt)
            else:
                nc.vector.memset(detT_new, 0.0)
            nc.sync.dma_start(out=detT_d[l][t], in_=detT_new)
            if l == 0:
                dest = out_f[:, t, 0:D].rearrange("b (q f) -> q b f", q=NQ)
                for q in range(NQ):
                    nc.sync.dma_start(out=dest[q], in_=detf_new[32 * q:32 * (q + 1), :])

            # --- qh
            pqh = None
            if detbf is not None:
                pqh = pps.tile([128, QD], F32, tag="p_qh", name="pqh")
                for j in range(NQ):
                    for k in range(KB):
                        nc.tensor.matmul(pqh[32 * j:32 * (j + 1), :], lhs_blk(detT_new, k),
                                         wqd[:, k, j, :], start=(k == 0), stop=(k == KB - 1),
                                         tile_position=(0, 32 * j))
            qhbf = work.tile([128, QD], BF16, tag="qhbf")
            if pqh is not None:
                q1 = work.tile([128, QD], F32, tag="q1")
                nc.vector.tensor_add(q1, pqh, opt)
                nc.vector.tensor_scalar_max(qhbf, q1, 0.0)
            else:
                nc.vector.tensor_scalar_max(qhbf, opt, 0.0)
            # --- T(qh)
            qhT_new = cpool.tile([128, 2, 128], BF16, tag="qhT")
            for s in range(2):
                pt = pps.tile([128, 128], BF16, tag="p_t", bufs=2, name="pt_q")
                nc.tensor.transpose(pt, qhbf[:, 128 * s:128 * (s + 1)], ident)
                nc.vector.tensor_copy(qhT_new[:, s, :], pt)
            if l == 0:
                nc.sync.dma_start(out=qhT_d[t], in_=qhT_new)

            detf_c, detT_c, qhT_c = detf_new, detT_new, qhT_new

    # ---------- postpass: pmean/pstd/qmean/qstd for level 0 ----------
    wpost = wpool.tile([128, 4, KB, S], BF16, tag="wfuse", name="wpost")
    for h in range(4):
        for k in range(KB):
            nc.gpsimd.dma_start(out=wpost[:, h, k, :], in_=ins["wpost"][h, k])
    T = Ts[0]
    for m in range(T // 4):
        # stage transposed det/qh for 4 timesteps: [128, q, s, t, b] so that for
        # each k-block=(q,s) the (t,b) free dims are contiguous (single free dim)
        dT4 = work.tile([128, NQ, 2, 4, 32], BF16, tag="dT4")
        qT4 = work.tile([128, NQ, 2, 4, 32], BF16, tag="qT4")
        for q in range(NQ):
            for s in range(2):
                nc.sync.dma_start(
                    out=dT4[:, q, s],
                    in_=detT_d[0][4 * m:4 * (m + 1), :, s, 32 * q:32 * (q + 1)]
                    .rearrange("t p b -> p t b"))
                nc.sync.dma_start(
                    out=qT4[:, q, s],
                    in_=qhT_d[4 * m:4 * (m + 1), :, s, 32 * q:32 * (q + 1)]
                    .rearrange("t p b -> p t b"))
        # heads: 0=pmean(det) 1=pstd(det) 2=qmean(qh) 3=qstd(qh)
        for h, (src, ptag) in enumerate([(dT4, "p_qh"), (dT4, "p_gin"), (qT4, "p_ghn"), (qT4, "p_rz")]):
            ph = pps.tile([128, S], F32, tag=ptag, name=f"post{h}")
            for k in range(KB):
                s, q = k % 2, k // 2
                lhsT = src[:, q, s].rearrange("p a b -> p (a b)")
                nc.tensor.matmul(ph, lhsT, wpost[:, h, k, :],
                                 start=(k == 0), stop=(k == KB - 1))
            hsb = work.tile([128, S], F32, tag="hsb")
            if h in (1, 3):
                # softplus(x + 0.54) + MIN_STD == ln(exp(x + 0.54) + 1) + MIN_STD
                he = work.tile([128, S], F32, tag="he")
                nc.scalar.activation(he, ph, mybir.ActivationFunctionType.Exp, bias=sp_bias)
                hl = work.tile([128, S], F32, tag="hl")
                nc.scalar.activation(hl, he, mybir.ActivationFunctionType.Ln, bias=1.0)
                nc.vector.tensor_scalar_add(hsb, hl, MIN_STD)
            else:
                nc.vector.tensor_copy(hsb, ph)
            for i in range(4):
                nc.sync.dma_start(out=out_f[:, 4 * m + i, D + S * h:D + S * (h + 1)],
                                  in_=hsb[32 * i:32 * (i + 1), :])




# ------------------------- runner -------------------------
_CACHE = {}


def _get_program(T0):
    if T0 in _CACHE:
        return _CACHE[T0]
    from concourse import bacc
    nc = bacc.Bacc("TRN2", target_bir_lowering=False, debug=False, num_devices=1)
    in_specs = _input_specs(T0)
    ins = {k: nc.dram_tensor(k, list(shape), dt, kind="ExternalInput").ap()
           for k, (shape, dt) in in_specs.items()}
    outs = {"out_f": nc.dram_tensor("out_f", [B, T0, D + 4 * S], F32,
                                    kind="ExternalOutput").ap()}
    with tile.TileContext(nc) as tc:
        with ExitStack() as ctx:
            build(ctx, tc, outs, ins, T0=T0)
    nc.compile()
    _CACHE[T0] = nc
    return nc


def _input_specs(T0):
    Ts = [T0, T0 // 4, T0 // 16]
    sp = {}
    for l in range(3):
        sp[f"wihrz{l}"] = ([KB, NQ, 128, 2 * QD], BF16)
        sp[f"wihn{l}"] = ([KB, NQ, 128, QD], BF16)
        sp[f"whhrz{l}"] = ([KB, NQ, 128, 2 * QD], BF16)
        sp[f"whhn{l}"] = ([KB, NQ, 128, QD], BF16)
        sp[f"wqd{l}"] = ([KB, NQ, 128, QD], BF16)
        sp[f"wfuse{l}"] = ([KB, NQ, 128, QD], BF16)
        sp[f"wqo{l}"] = ([KB, 128, E], BF16)
        if l < 2:
            sp[f"wctx{l}"] = ([KB, NQ, 128, QD], BF16)
        sp[f"obs{l}"] = ([B, Ts[l], E], BF16)
    sp["wpost"] = ([4, KB, 128, S], BF16)
    return sp


def run(inputs, trace=False):
    from concourse.bass_utils import run_bass_kernel_spmd
    T0 = int(inputs["obs_l0"].shape[1])
    prepped = prep_inputs(inputs, T0)
    nc = _get_program(T0)
    res = run_bass_kernel_spmd(nc, [prepped], core_ids=[0], trace=trace)
    out = res.results[0]["out_f"].astype(np.float32)
    return out, res


def kernel(**inputs):
    out, _ = run(inputs, trace=False)
    return out
